# revision 1
# baseline (speedup 1.0000x reference)
"""Bidirectional GRU encoder (V=32000, H=512, T=512, B=64) on 8 Trainium2 NeuronCores.

Strategy (uniform SPMD program, asymmetry fed via per-core data):
  - core c in 0..3: forward direction, batch slice [16c, 16c+16)
  - core c in 4..7: backward direction, same slices, fed a time-reversed
    sequence (the reference's per-sequence double reversal is algebraically a
    reverse-order masked scan, so the program is identical).
  Per core: indirect-DMA embedding gather -> PE transpose to H-major ->
  float32r matmuls for GI = emb @ Wih.T (+bias), stored fp16 -> 512-step
  recurrence with fp16 weights-stationary matmuls, gates on ACT/DVE,
  state-freeze via z = max(z, notm) -> pairwise AllGather (c <-> c+4) ->
  on-device combine mask*(out_f + out_b) -> host reassembles layouts.
"""
import sys

sys.path.insert(0, "/opt/trn_rl_repo")

import numpy as np

import concourse.bacc as bacc
import concourse.bass as bass
import concourse.mybir as mybir
from concourse.masks import make_identity
from concourse.tile import TileContext

P = 128
H = 512
KT = H // P          # 4 k-tiles over the hidden dim
G3 = 3 * H           # 1536 gate columns (r, z, n)
MT = G3 // P         # 12 m-tiles
B_LOC = 16           # batch lanes per core
NCORES = 8
V = 32000

F32 = mybir.dt.float32
F32R = mybir.dt.float32r
F16 = mybir.dt.float16
I32 = mybir.dt.int32

AF = mybir.ActivationFunctionType
OP = mybir.AluOpType


def build_program(T):
    ROWS = T * B_LOC
    NRT = ROWS // P          # 128-row gather tiles
    NCH = NRT // 4           # gi chunks of 512 rows (32 t's)
    assert NRT % 4 == 0 and T % 4 == 0

    nc = bacc.Bacc("TRN2", target_bir_lowering=False, num_devices=NCORES)

    emb = nc.declare_dram_parameter("emb", [V, H], F32, isOutput=False)
    seq = nc.declare_dram_parameter("seq", [ROWS], I32, isOutput=False)
    wih = nc.declare_dram_parameter("wih", [P, KT, G3], F32, isOutput=False)
    whh = nc.declare_dram_parameter("whh", [P, KT, G3], F16, isOutput=False)
    brz = nc.declare_dram_parameter("brz", [P, 8], F32, isOutput=False)
    bn = nc.declare_dram_parameter("bn", [P, 4], F32, isOutput=False)
    bhn = nc.declare_dram_parameter("bhn", [P, 4], F32, isOutput=False)
    notmb = nc.declare_dram_parameter("notmb", [T, P, B_LOC], F16, isOutput=False)
    maskb = nc.declare_dram_parameter("maskb", [T, P, B_LOC], F16, isOutput=False)

    gi = nc.dram_tensor("gi", [T, MT, P, B_LOC], F16)
    raw = nc.dram_tensor("raw", [T, KT, P, B_LOC], F16)
    gath = nc.dram_tensor("gath", [2, T, KT, P, B_LOC], F16)

    outc = nc.declare_dram_parameter("outc", [T, KT, P, B_LOC], F32, isOutput=True)
    hlast = nc.declare_dram_parameter("hlast", [KT, P, B_LOC], F32, isOutput=True)

    with TileContext(nc) as tc:
        with (
            tc.tile_pool(name="const", bufs=1) as cpool,
            tc.tile_pool(name="gis", bufs=8) as gpool,
            tc.tile_pool(name="work", bufs=3) as wkpool,
            tc.tile_pool(name="hbuf", bufs=2) as hpool,
            tc.tile_pool(name="emb", bufs=2) as epool,
            tc.tile_pool(name="psA", bufs=2, space="PSUM") as psA,
            tc.tile_pool(name="psB", bufs=2, space="PSUM") as psB,
        ):
            # ---- Phase A: constants / weights
            ident = cpool.tile([P, P], F32)
            make_identity(nc, ident[:])

            whh_sb = cpool.tile([P, KT, G3], F16)
            nc.sync.dma_start(out=whh_sb[:], in_=whh[:])
            wih_raw = cpool.tile([P, KT, G3], F32)
            nc.sync.dma_start(out=wih_raw[:], in_=wih[:])
            wih_sb = cpool.tile([P, KT, G3], F32R)
            nc.vector.tensor_copy(wih_sb[:], wih_raw[:])

            brz_sb = cpool.tile([P, 8], F32)
            nc.sync.dma_start(out=brz_sb[:], in_=brz[:])
            bn_sb = cpool.tile([P, 4], F32)
            nc.sync.dma_start(out=bn_sb[:], in_=bn[:])
            bhn_sb = cpool.tile([P, 4], F32)
            nc.sync.dma_start(out=bhn_sb[:], in_=bhn[:])

            seq_sb = cpool.tile([P, NRT], I32)
            nc.sync.dma_start(out=seq_sb[:], in_=seq[:].rearrange("(n p) -> p n", p=P))

            # ---- Phase B: GI = emb_rows @ Wih.T + bias   (f32r, N=512 chunks)
            for ch in range(NCH):
                embT = epool.tile([P, KT, 512], F32R, tag="embT")
                for rt in range(4):
                    col = ch * 4 + rt
                    rows_sb = epool.tile([P, H], F32, tag="rows")
                    nc.gpsimd.indirect_dma_start(
                        out=rows_sb[:],
                        out_offset=None,
                        in_=emb[:],
                        in_offset=bass.IndirectOffsetOnAxis(
                            ap=seq_sb[:, col : col + 1], axis=0
                        ),
                    )
                    for k in range(KT):
                        tp = psA.tile([P, P], F32, space="PSUM", tag="tpsum")
                        nc.tensor.transpose(
                            tp[:], rows_sb[:, k * P : (k + 1) * P], ident[:]
                        )
                        nc.vector.tensor_copy(
                            embT[:, k, rt * P : (rt + 1) * P], tp[:]
                        )
                for m in range(MT):
                    pm = psB.tile([P, 512], F32, space="PSUM", tag="gials")
                    for k in range(KT):
                        nc.tensor.matmul(
                            out=pm[:],
                            lhsT=wih_sb[:, k, m * P : (m + 1) * P],
                            rhs=embT[:, k, :],
                            start=(k == 0),
                            stop=(k == KT - 1),
                        )
                    gi_sb = wkpool.tile([P, 512], F16, tag="gisb")
                    bias_ap = (
                        brz_sb[:, m : m + 1] if m < 8 else bn_sb[:, m - 8 : m - 7]
                    )
                    nc.scalar.activation(
                        gi_sb[:], pm[:], AF.Identity, bias=bias_ap, scale=1.0
                    )
                    nc.sync.dma_start(
                        out=gi[ch * (T // NCH) : (ch + 1) * (T // NCH), m].rearrange(
                            "t p b -> p t b"
                        ),
                        in_=gi_sb[:].rearrange("p (t b) -> p t b", b=B_LOC),
                    )

            # ---- Phase C: the recurrence
            h_prev = hpool.tile([P, KT, B_LOC], F16, tag="h")
            nc.vector.memset(h_prev[:], 0.0)

            for t in range(T):
                gis = gpool.tile([P, MT, B_LOC], F16, tag="gis")
                nc.sync.dma_start(out=gis[:], in_=gi[t].rearrange("m p b -> p m b"))
                ntm = gpool.tile([P, B_LOC], F16, tag="ntm")
                nc.sync.dma_start(out=ntm[:], in_=notmb[t])

                h_new = hpool.tile([P, KT, B_LOC], F16, tag="h")
                gig = gis[:].rearrange("p (g m) b -> p g m b", g=3)

                for c in (0, 1):
                    ms = [2 * c, 2 * c + 1, 4 + 2 * c, 5 + 2 * c, 8 + 2 * c, 9 + 2 * c]
                    ps = psA.tile([P, 6 * B_LOC], F32, space="PSUM", tag=f"rec{c}")
                    for j, m in enumerate(ms):
                        wslice = whh_sb[:, :, m * P : (m + 1) * P]
                        for k in range(KT):
                            nc.tensor.ldweights(weights=wslice[:, k, :])
                            nc.tensor.matmul(
                                out=ps[:, j * B_LOC : (j + 1) * B_LOC],
                                lhsT=wslice[:, k, :],
                                rhs=h_prev[:, k, :],
                                start=(k == 0),
                                stop=(k == KT - 1),
                            )
                    # gates for H-slices {2c, 2c+1}
                    ps_rz = ps[:, 0 : 4 * B_LOC].rearrange(
                        "p (g m b) -> p g m b", g=2, m=2
                    )
                    ps_n = ps[:, 4 * B_LOC : 6 * B_LOC].rearrange(
                        "p (m b) -> p m b", m=2
                    )
                    s_rz = wkpool.tile([P, 2, 2, B_LOC], F16, tag=f"srz{c}")
                    nc.vector.tensor_tensor(
                        out=s_rz[:], in0=ps_rz, in1=gig[:, 0:2, 2 * c : 2 * c + 2, :],
                        op=OP.add,
                    )
                    rz = wkpool.tile([P, 2, 2, B_LOC], F16, tag=f"rz{c}")
                    nc.scalar.activation(rz[:], s_rz[:], AF.Sigmoid)
                    r_c = rz[:, 0]
                    zf = wkpool.tile([P, 2, B_LOC], F16, tag=f"zf{c}")
                    nc.vector.tensor_tensor(
                        out=zf[:], in0=rz[:, 1],
                        in1=ntm[:, None, :].to_broadcast([P, 2, B_LOC]),
                        op=OP.max,
                    )
                    pn = wkpool.tile([P, 2, B_LOC], F16, tag=f"pn{c}")
                    for j in range(2):
                        nc.vector.tensor_scalar(
                            out=pn[:, j], in0=ps_n[:, j],
                            scalar1=bhn_sb[:, 2 * c + j : 2 * c + j + 1],
                            scalar2=None, op0=OP.add,
                        )
                    pr = wkpool.tile([P, 2, B_LOC], F16, tag=f"pr{c}")
                    nc.vector.tensor_tensor(out=pr[:], in0=r_c, in1=pn[:], op=OP.mult)
                    pre = wkpool.tile([P, 2, B_LOC], F16, tag=f"pre{c}")
                    nc.vector.tensor_tensor(
                        out=pre[:], in0=pr[:], in1=gig[:, 2, 2 * c : 2 * c + 2, :],
                        op=OP.add,
                    )
                    n_t = wkpool.tile([P, 2, B_LOC], F16, tag=f"nt{c}")
                    nc.scalar.activation(n_t[:], pre[:], AF.Tanh)
                    zh = wkpool.tile([P, 2, B_LOC], F16, tag=f"zh{c}")
                    nc.vector.tensor_tensor(
                        out=zh[:], in0=zf[:], in1=h_prev[:, 2 * c : 2 * c + 2, :],
                        op=OP.mult,
                    )
                    w_t = wkpool.tile([P, 2, B_LOC], F16, tag=f"wt{c}")
                    nc.vector.tensor_scalar(
                        out=w_t[:], in0=zf[:], scalar1=-1.0, scalar2=1.0,
                        op0=OP.mult, op1=OP.add,
                    )
                    u_t = wkpool.tile([P, 2, B_LOC], F16, tag=f"ut{c}")
                    nc.vector.tensor_tensor(out=u_t[:], in0=w_t[:], in1=n_t[:], op=OP.mult)
                    nc.vector.tensor_tensor(
                        out=h_new[:, 2 * c : 2 * c + 2, :], in0=zh[:], in1=u_t[:],
                        op=OP.add,
                    )

                nc.sync.dma_start(
                    out=raw[t].rearrange("k p b -> p k b"), in_=h_new[:]
                )
                h_prev = h_new

            # ---- Phase D: hidden out, exchange, combine
            hf = wkpool.tile([P, KT, B_LOC], F32, tag="hf32")
            nc.vector.tensor_copy(hf[:], h_prev[:])
            nc.sync.dma_start(out=hlast[:].rearrange("k p b -> p k b"), in_=hf[:])

            nc.gpsimd.collective_compute(
                "AllGather",
                OP.bypass,
                replica_groups=[[i, i + 4] for i in range(4)],
                ins=[raw[:]],
                outs=[gath[:]],
            )

            for g in range(T // 4):
                t0 = g * 4
                f_sb = wkpool.tile([P, 4, KT, B_LOC], F16, tag="fsb")
                nc.sync.dma_start(
                    out=f_sb[:],
                    in_=gath[0, t0 : t0 + 4].rearrange("t k p b -> p t k b"),
                )
                b_sb = wkpool.tile([P, 4, KT, B_LOC], F16, tag="bsb")
                for i in range(4):
                    nc.sync.dma_start(
                        out=b_sb[:, i],
                        in_=gath[1, T - 1 - (t0 + i)].rearrange("k p b -> p k b"),
                    )
                m_sb = wkpool.tile([P, 4, B_LOC], F16, tag="msb")
                nc.sync.dma_start(
                    out=m_sb[:], in_=maskb[t0 : t0 + 4].rearrange("t p b -> p t b")
                )
                s_sb = wkpool.tile([P, 4, KT, B_LOC], F16, tag="ssb")
                nc.vector.tensor_tensor(out=s_sb[:], in0=f_sb[:], in1=b_sb[:], op=OP.add)
                o_sb = wkpool.tile([P, 4, KT, B_LOC], F32, tag="osb")
                nc.vector.tensor_tensor(
                    out=o_sb[:], in0=s_sb[:],
                    in1=m_sb[:, :, None, :].to_broadcast([P, 4, KT, B_LOC]),
                    op=OP.mult,
                )
                nc.sync.dma_start(
                    out=outc[t0 : t0 + 4].rearrange("t k p b -> p t k b"),
                    in_=o_sb[:],
                )

    nc.finalize()
    return nc


def host_prep(T, sequence, input_lengths, embedding,
              Wih_f, Whh_f, bih_f, bhh_f, Wih_b, Whh_b, bih_b, bhh_b):
    """Build the 8 per-core input maps."""
    sequence = np.asarray(sequence)
    input_lengths = np.asarray(input_lengths)
    embedding = np.ascontiguousarray(np.asarray(embedding, dtype=np.float32))

    t_idx = np.arange(T)[:, None]
    mask_full = (t_idx < input_lengths[None, :])          # [T, B] real time

    def wprep(Wih, Whh, bih, bhh):
        wihT = np.ascontiguousarray(
            np.asarray(Wih, np.float32).T.reshape(KT, P, G3).transpose(1, 0, 2)
        )
        whhT = np.ascontiguousarray(
            np.asarray(Whh, np.float16).T.reshape(KT, P, G3).transpose(1, 0, 2)
        )
        b = np.asarray(bih, np.float32) + np.asarray(bhh, np.float32)
        brz_v = np.ascontiguousarray(b[: 8 * P].reshape(8, P).T)
        bn_v = np.ascontiguousarray(np.asarray(bih, np.float32)[8 * P :].reshape(4, P).T)
        bhn_v = np.ascontiguousarray(np.asarray(bhh, np.float32)[8 * P :].reshape(4, P).T)
        return wihT, whhT, brz_v, bn_v, bhn_v

    wf = wprep(Wih_f, Whh_f, bih_f, bhh_f)
    wb = wprep(Wih_b, Whh_b, bih_b, bhh_b)

    in_maps = []
    for c in range(NCORES):
        d, s = c // 4, c % 4
        bsl = slice(B_LOC * s, B_LOC * (s + 1))
        seq_c = sequence[:, bsl].astype(np.int32)
        m_loc = mask_full[:, bsl]
        if d == 1:
            seq_c = seq_c[::-1]
            m_loc = m_loc[::-1]
        notm = (~m_loc).astype(np.float16)                   # local time
        maskc = mask_full[:, bsl].astype(np.float16)         # real time
        wihT, whhT, brz_v, bn_v, bhn_v = wf if d == 0 else wb
        in_maps.append(
            dict(
                emb=embedding,
                seq=np.ascontiguousarray(seq_c.reshape(-1)),
                wih=wihT, whh=whhT, brz=brz_v, bn=bn_v, bhn=bhn_v,
                notmb=np.ascontiguousarray(
                    np.broadcast_to(notm[:, None, :], (T, P, B_LOC))
                ),
                maskb=np.ascontiguousarray(
                    np.broadcast_to(maskc[:, None, :], (T, P, B_LOC))
                ),
            )
        )
    return in_maps


class CompiledKernel:
    """Compiles the SPMD program once and keeps the sharded jit callable."""

    def __init__(self, T):
        import jax
        from jax.sharding import Mesh, PartitionSpec
        from jax.experimental.shard_map import shard_map
        from concourse import bass2jax
        from concourse.bass2jax import _bass_exec_p, partition_id_tensor, install_neuronx_cc_hook

        self.T = T
        nc = build_program(T)
        self.nc = nc
        install_neuronx_cc_hook()

        partition_name = nc.partition_id_tensor.name if nc.partition_id_tensor else None
        in_names, out_names, out_avals = [], [], []
        zero_outs = []
        for alloc in nc.m.functions[0].allocations:
            if not isinstance(alloc, mybir.MemoryLocationSet):
                continue
            name = alloc.memorylocations[0].name
            if alloc.kind == "ExternalInput":
                if name != partition_name:
                    in_names.append(name)
            elif alloc.kind == "ExternalOutput":
                shape = tuple(alloc.tensor_shape)
                dtype = mybir.dt.np(alloc.dtype)
                out_names.append(name)
                out_avals.append(jax.core.ShapedArray(shape, dtype))
                zero_outs.append(np.zeros(shape, dtype))
        self.n_params = len(in_names)
        self.param_names = list(in_names)
        self.out_names = list(out_names)
        self.zero_outs = zero_outs
        in_names = in_names + out_names
        if partition_name is not None:
            in_names.append(partition_name)

        def _body(*args):
            operands = list(args)
            if partition_name is not None:
                operands.append(partition_id_tensor())
            outs = _bass_exec_p.bind(
                *operands,
                out_avals=tuple(out_avals),
                in_names=tuple(in_names),
                out_names=tuple(out_names),
                lowering_input_output_aliases=(),
                sim_require_finite=True,
                sim_require_nnan=True,
                nc=nc,
            )
            return tuple(outs)

        devices = jax.devices()[:NCORES]
        self.mesh = Mesh(np.asarray(devices), ("core",))
        n_outs = len(out_names)
        in_specs = (PartitionSpec("core"),) * (self.n_params + n_outs)
        out_specs = (PartitionSpec("core"),) * n_outs
        donate = tuple(range(self.n_params, self.n_params + n_outs))
        self.sharded = jax.jit(
            shard_map(_body, mesh=self.mesh, in_specs=in_specs, out_specs=out_specs,
                      check_rep=False),
            donate_argnums=donate, keep_unused=True,
        )
        self.out_avals = out_avals
        self._concat_in = None

    def stage_inputs(self, in_maps):
        self._concat_in = [
            np.concatenate([np.asarray(in_maps[c][n]) for c in range(NCORES)], axis=0)
            for n in self.param_names
        ]

    def run(self):
        import jax
        zeros = [np.zeros((NCORES * z.shape[0], *z.shape[1:]), z.dtype)
                 for z in self.zero_outs]
        out = self.sharded(*self._concat_in, *zeros)
        out = jax.block_until_ready(out)
        return [
            {
                name: np.asarray(out[i]).reshape(NCORES, *self.out_avals[i].shape)[c]
                for i, name in enumerate(self.out_names)
            }
            for c in range(NCORES)
        ]


_CACHE = {}


def _get_compiled(T):
    if T not in _CACHE:
        _CACHE[T] = CompiledKernel(T)
    return _CACHE[T]


def assemble(T, results):
    B = B_LOC * 4
    outputs = np.empty((T, B, H), np.float32)
    hidden = np.empty((2, B, H), np.float32)
    half = T // 2
    for c in range(NCORES):
        d, s = c // 4, c % 4
        bsl = slice(B_LOC * s, B_LOC * (s + 1))
        oc = results[c]["outc"]                       # [T, KT, P, B_LOC]
        arr = oc.transpose(0, 3, 1, 2).reshape(T, B_LOC, H)
        if d == 0:
            outputs[:half, bsl] = arr[:half]
        else:
            outputs[half:, bsl] = arr[half:]
        hl = results[c]["hlast"]                      # [KT, P, B_LOC]
        hidden[d, bsl] = hl.transpose(2, 0, 1).reshape(B_LOC, H)
    return outputs, hidden


def kernel(sequence, input_lengths, embedding,
           Wih_f, Whh_f, bih_f, bhh_f, Wih_b, Whh_b, bih_b, bhh_b):
    T = int(np.asarray(sequence).shape[0])
    ck = _get_compiled(T)
    in_maps = host_prep(T, sequence, input_lengths, embedding,
                        Wih_f, Whh_f, bih_f, bhh_f, Wih_b, Whh_b, bih_b, bhh_b)
    ck.stage_inputs(in_maps)
    results = ck.run()
    return assemble(T, results)


# revision 2
# speedup vs baseline: 4.5490x; 4.5490x over previous
"""Bidirectional GRU encoder (V=32000, H=512, T=512, B=64) on 8 Trainium2 NeuronCores.

Strategy (uniform SPMD program, asymmetry fed via per-core data):
  - core c in 0..3: forward direction, batch slice [16c, 16c+16)
  - core c in 4..7: backward direction, same slices, fed a time-reversed
    sequence (the reference's per-sequence double reversal is algebraically a
    reverse-order masked scan, so the program is identical).
  Per core: indirect-DMA embedding gather -> PE transpose to H-major ->
  float32r matmuls for GI = emb @ Wih.T (+bias), stored fp16 -> 512-step
  recurrence with fp16 weights-stationary matmuls, gates on ACT/DVE,
  state-freeze via z = max(z, notm) -> pairwise AllGather (c <-> c+4) ->
  on-device combine mask*(out_f + out_b) -> host reassembles layouts.
"""
import sys

sys.path.insert(0, "/opt/trn_rl_repo")

import numpy as np

import concourse.bacc as bacc
import concourse.bass as bass
import concourse.mybir as mybir
from concourse.masks import make_identity
from concourse.tile import TileContext

P = 128
H = 512
KT = H // P          # 4 k-tiles over the hidden dim
G3 = 3 * H           # 1536 gate columns (r, z, n)
MT = G3 // P         # 12 m-tiles
B_LOC = 16           # batch lanes per core
NCORES = 8
V = 32000

F32 = mybir.dt.float32
F32R = mybir.dt.float32r
F16 = mybir.dt.float16
I32 = mybir.dt.int32

AF = mybir.ActivationFunctionType
OP = mybir.AluOpType


def build_program(T):
    ROWS = T * B_LOC
    NRT = ROWS // P          # 128-row gather tiles
    NCH = NRT // 4           # gi chunks of 512 rows (32 t's)
    assert NRT % 4 == 0 and T % 4 == 0

    nc = bacc.Bacc("TRN2", target_bir_lowering=False, num_devices=NCORES)

    emb = nc.declare_dram_parameter("emb", [V, H], F32, isOutput=False)
    seq = nc.declare_dram_parameter("seq", [ROWS], I32, isOutput=False)
    wih = nc.declare_dram_parameter("wih", [P, KT, G3], F32, isOutput=False)
    whh = nc.declare_dram_parameter("whh", [P, KT, G3], F16, isOutput=False)
    brz = nc.declare_dram_parameter("brz", [P, 8], F32, isOutput=False)
    bn = nc.declare_dram_parameter("bn", [P, 4], F32, isOutput=False)
    bhn = nc.declare_dram_parameter("bhn", [P, 4], F32, isOutput=False)
    notmb = nc.declare_dram_parameter("notmb", [T, P, B_LOC], F16, isOutput=False)
    maskb = nc.declare_dram_parameter("maskb", [T, P, B_LOC], F16, isOutput=False)

    gi = nc.dram_tensor("gi", [T, MT, P, B_LOC], F16)
    raw = nc.dram_tensor("raw", [T, KT, P, B_LOC], F16)
    gath = nc.dram_tensor("gath", [2, T, KT, P, B_LOC], F16)

    outc = nc.declare_dram_parameter("outc", [T, KT, P, B_LOC], F32, isOutput=True)
    hlast = nc.declare_dram_parameter("hlast", [KT, P, B_LOC], F32, isOutput=True)

    with TileContext(nc) as tc:
        with (
            tc.tile_pool(name="const", bufs=1) as cpool,
            tc.tile_pool(name="gis", bufs=8) as gpool,
            tc.tile_pool(name="work", bufs=3) as wkpool,
            tc.tile_pool(name="hbuf", bufs=2) as hpool,
            tc.tile_pool(name="emb", bufs=2) as epool,
            tc.tile_pool(name="psA", bufs=2, space="PSUM") as psA,
            tc.tile_pool(name="psB", bufs=2, space="PSUM") as psB,
        ):
            # ---- Phase A: constants / weights
            ident = cpool.tile([P, P], F32)
            make_identity(nc, ident[:])

            whh_sb = cpool.tile([P, KT, G3], F16)
            nc.sync.dma_start(out=whh_sb[:], in_=whh[:])
            wih_raw = cpool.tile([P, KT, G3], F32)
            nc.sync.dma_start(out=wih_raw[:], in_=wih[:])
            wih_sb = cpool.tile([P, KT, G3], F32R)
            nc.vector.tensor_copy(wih_sb[:], wih_raw[:])

            brz_sb = cpool.tile([P, 8], F32)
            nc.sync.dma_start(out=brz_sb[:], in_=brz[:])
            bn_sb = cpool.tile([P, 4], F32)
            nc.sync.dma_start(out=bn_sb[:], in_=bn[:])
            bhn_sb = cpool.tile([P, 4], F32)
            nc.sync.dma_start(out=bhn_sb[:], in_=bhn[:])

            seq_sb = cpool.tile([P, NRT], I32)
            nc.sync.dma_start(out=seq_sb[:], in_=seq[:].rearrange("(n p) -> p n", p=P))

            # ---- Phase B: GI = emb_rows @ Wih.T + bias   (f32r, N=512 chunks)
            for ch in range(NCH):
                embT = epool.tile([P, KT, 512], F32R, tag="embT")
                for rt in range(4):
                    col = ch * 4 + rt
                    rows_sb = epool.tile([P, H], F32, tag="rows")
                    nc.gpsimd.indirect_dma_start(
                        out=rows_sb[:],
                        out_offset=None,
                        in_=emb[:],
                        in_offset=bass.IndirectOffsetOnAxis(
                            ap=seq_sb[:, col : col + 1], axis=0
                        ),
                    )
                    for k in range(KT):
                        tp = psA.tile([P, P], F32, space="PSUM", tag="tpsum")
                        nc.tensor.transpose(
                            tp[:], rows_sb[:, k * P : (k + 1) * P], ident[:]
                        )
                        nc.vector.tensor_copy(
                            embT[:, k, rt * P : (rt + 1) * P], tp[:]
                        )
                for m in range(MT):
                    pm = psB.tile([P, 512], F32, space="PSUM", tag="gials")
                    for k in range(KT):
                        nc.tensor.matmul(
                            out=pm[:],
                            lhsT=wih_sb[:, k, m * P : (m + 1) * P],
                            rhs=embT[:, k, :],
                            start=(k == 0),
                            stop=(k == KT - 1),
                        )
                    gi_sb = wkpool.tile([P, 512], F16, tag="gisb")
                    bias_ap = (
                        brz_sb[:, m : m + 1] if m < 8 else bn_sb[:, m - 8 : m - 7]
                    )
                    nc.scalar.activation(
                        gi_sb[:], pm[:], AF.Identity, bias=bias_ap, scale=1.0
                    )
                    nc.sync.dma_start(
                        out=gi[ch * (T // NCH) : (ch + 1) * (T // NCH), m].rearrange(
                            "t p b -> p t b"
                        ),
                        in_=gi_sb[:].rearrange("p (t b) -> p t b", b=B_LOC),
                    )

            # ---- Phase C: the recurrence
            h_prev = hpool.tile([P, KT, B_LOC], F16, tag="h")
            nc.vector.memset(h_prev[:], 0.0)

            for t in range(T):
                gis = gpool.tile([P, MT, B_LOC], F16, tag="gis")
                nc.sync.dma_start(out=gis[:], in_=gi[t].rearrange("m p b -> p m b"))
                ntm = gpool.tile([P, B_LOC], F16, tag="ntm")
                nc.sync.dma_start(out=ntm[:], in_=notmb[t])

                h_new = hpool.tile([P, KT, B_LOC], F16, tag="h")
                gig = gis[:].rearrange("p (g m) b -> p g m b", g=3)

                for c in (0, 1):
                    ms = [2 * c, 2 * c + 1, 4 + 2 * c, 5 + 2 * c, 8 + 2 * c, 9 + 2 * c]
                    ps = psA.tile([P, 6 * B_LOC], F32, space="PSUM", tag=f"rec{c}")
                    for j, m in enumerate(ms):
                        wslice = whh_sb[:, :, m * P : (m + 1) * P]
                        for k in range(KT):
                            nc.tensor.ldweights(weights=wslice[:, k, :])
                            nc.tensor.matmul(
                                out=ps[:, j * B_LOC : (j + 1) * B_LOC],
                                lhsT=wslice[:, k, :],
                                rhs=h_prev[:, k, :],
                                start=(k == 0),
                                stop=(k == KT - 1),
                            )
                    # gates for H-slices {2c, 2c+1}
                    ps_rz = ps[:, 0 : 4 * B_LOC].rearrange(
                        "p (g m b) -> p g m b", g=2, m=2
                    )
                    ps_n = ps[:, 4 * B_LOC : 6 * B_LOC].rearrange(
                        "p (m b) -> p m b", m=2
                    )
                    s_rz = wkpool.tile([P, 2, 2, B_LOC], F16, tag=f"srz{c}")
                    nc.vector.tensor_tensor(
                        out=s_rz[:], in0=ps_rz, in1=gig[:, 0:2, 2 * c : 2 * c + 2, :],
                        op=OP.add,
                    )
                    rz = wkpool.tile([P, 2, 2, B_LOC], F16, tag=f"rz{c}")
                    nc.scalar.activation(rz[:], s_rz[:], AF.Sigmoid)
                    r_c = rz[:, 0]
                    zf = wkpool.tile([P, 2, B_LOC], F16, tag=f"zf{c}")
                    nc.vector.tensor_tensor(
                        out=zf[:], in0=rz[:, 1],
                        in1=ntm[:, None, :].to_broadcast([P, 2, B_LOC]),
                        op=OP.max,
                    )
                    pn = wkpool.tile([P, 2, B_LOC], F16, tag=f"pn{c}")
                    for j in range(2):
                        nc.vector.tensor_scalar(
                            out=pn[:, j], in0=ps_n[:, j],
                            scalar1=bhn_sb[:, 2 * c + j : 2 * c + j + 1],
                            scalar2=None, op0=OP.add,
                        )
                    pr = wkpool.tile([P, 2, B_LOC], F16, tag=f"pr{c}")
                    nc.vector.tensor_tensor(out=pr[:], in0=r_c, in1=pn[:], op=OP.mult)
                    pre = wkpool.tile([P, 2, B_LOC], F16, tag=f"pre{c}")
                    nc.vector.tensor_tensor(
                        out=pre[:], in0=pr[:], in1=gig[:, 2, 2 * c : 2 * c + 2, :],
                        op=OP.add,
                    )
                    n_t = wkpool.tile([P, 2, B_LOC], F16, tag=f"nt{c}")
                    nc.scalar.activation(n_t[:], pre[:], AF.Tanh)
                    zh = wkpool.tile([P, 2, B_LOC], F16, tag=f"zh{c}")
                    nc.vector.tensor_tensor(
                        out=zh[:], in0=zf[:], in1=h_prev[:, 2 * c : 2 * c + 2, :],
                        op=OP.mult,
                    )
                    w_t = wkpool.tile([P, 2, B_LOC], F16, tag=f"wt{c}")
                    nc.vector.tensor_scalar(
                        out=w_t[:], in0=zf[:], scalar1=-1.0, scalar2=1.0,
                        op0=OP.mult, op1=OP.add,
                    )
                    u_t = wkpool.tile([P, 2, B_LOC], F16, tag=f"ut{c}")
                    nc.vector.tensor_tensor(out=u_t[:], in0=w_t[:], in1=n_t[:], op=OP.mult)
                    nc.vector.tensor_tensor(
                        out=h_new[:, 2 * c : 2 * c + 2, :], in0=zh[:], in1=u_t[:],
                        op=OP.add,
                    )

                nc.sync.dma_start(
                    out=raw[t].rearrange("k p b -> p k b"), in_=h_new[:]
                )
                h_prev = h_new

            # ---- Phase D: hidden out, exchange, combine
            hf = wkpool.tile([P, KT, B_LOC], F32, tag="hf32")
            nc.vector.tensor_copy(hf[:], h_prev[:])
            nc.sync.dma_start(out=hlast[:].rearrange("k p b -> p k b"), in_=hf[:])

            nc.gpsimd.collective_compute(
                "AllGather",
                OP.bypass,
                replica_groups=[[i, i + 4] for i in range(4)],
                ins=[raw[:]],
                outs=[gath[:]],
            )

            for g in range(T // 4):
                t0 = g * 4
                f_sb = wkpool.tile([P, 4, KT, B_LOC], F16, tag="fsb")
                nc.sync.dma_start(
                    out=f_sb[:],
                    in_=gath[0, t0 : t0 + 4].rearrange("t k p b -> p t k b"),
                )
                b_sb = wkpool.tile([P, 4, KT, B_LOC], F16, tag="bsb")
                for i in range(4):
                    nc.sync.dma_start(
                        out=b_sb[:, i],
                        in_=gath[1, T - 1 - (t0 + i)].rearrange("k p b -> p k b"),
                    )
                m_sb = wkpool.tile([P, 4, B_LOC], F16, tag="msb")
                nc.sync.dma_start(
                    out=m_sb[:], in_=maskb[t0 : t0 + 4].rearrange("t p b -> p t b")
                )
                s_sb = wkpool.tile([P, 4, KT, B_LOC], F16, tag="ssb")
                nc.vector.tensor_tensor(out=s_sb[:], in0=f_sb[:], in1=b_sb[:], op=OP.add)
                o_sb = wkpool.tile([P, 4, KT, B_LOC], F32, tag="osb")
                nc.vector.tensor_tensor(
                    out=o_sb[:], in0=s_sb[:],
                    in1=m_sb[:, :, None, :].to_broadcast([P, 4, KT, B_LOC]),
                    op=OP.mult,
                )
                nc.sync.dma_start(
                    out=outc[t0 : t0 + 4].rearrange("t k p b -> p t k b"),
                    in_=o_sb[:],
                )

    nc.finalize()
    return nc


def host_prep(T, sequence, input_lengths, embedding,
              Wih_f, Whh_f, bih_f, bhh_f, Wih_b, Whh_b, bih_b, bhh_b):
    """Build the 8 per-core input maps."""
    sequence = np.asarray(sequence)
    input_lengths = np.asarray(input_lengths)
    embedding = np.ascontiguousarray(np.asarray(embedding, dtype=np.float32))

    t_idx = np.arange(T)[:, None]
    mask_full = (t_idx < input_lengths[None, :])          # [T, B] real time

    def wprep(Wih, Whh, bih, bhh):
        wihT = np.ascontiguousarray(
            np.asarray(Wih, np.float32).T.reshape(KT, P, G3).transpose(1, 0, 2)
        )
        whhT = np.ascontiguousarray(
            np.asarray(Whh, np.float16).T.reshape(KT, P, G3).transpose(1, 0, 2)
        )
        b = np.asarray(bih, np.float32) + np.asarray(bhh, np.float32)
        brz_v = np.ascontiguousarray(b[: 8 * P].reshape(8, P).T)
        bn_v = np.ascontiguousarray(np.asarray(bih, np.float32)[8 * P :].reshape(4, P).T)
        bhn_v = np.ascontiguousarray(np.asarray(bhh, np.float32)[8 * P :].reshape(4, P).T)
        return wihT, whhT, brz_v, bn_v, bhn_v

    wf = wprep(Wih_f, Whh_f, bih_f, bhh_f)
    wb = wprep(Wih_b, Whh_b, bih_b, bhh_b)

    in_maps = []
    for c in range(NCORES):
        d, s = c // 4, c % 4
        bsl = slice(B_LOC * s, B_LOC * (s + 1))
        seq_c = sequence[:, bsl].astype(np.int32)
        m_loc = mask_full[:, bsl]
        if d == 1:
            seq_c = seq_c[::-1]
            m_loc = m_loc[::-1]
        notm = (~m_loc).astype(np.float16)                   # local time
        maskc = mask_full[:, bsl].astype(np.float16)         # real time
        wihT, whhT, brz_v, bn_v, bhn_v = wf if d == 0 else wb
        in_maps.append(
            dict(
                emb=embedding,
                seq=np.ascontiguousarray(seq_c.reshape(-1)),
                wih=wihT, whh=whhT, brz=brz_v, bn=bn_v, bhn=bhn_v,
                notmb=np.ascontiguousarray(
                    np.broadcast_to(notm[:, None, :], (T, P, B_LOC))
                ),
                maskb=np.ascontiguousarray(
                    np.broadcast_to(maskc[:, None, :], (T, P, B_LOC))
                ),
            )
        )
    return in_maps


class CompiledKernel:
    """Compiles the SPMD program once and keeps the sharded jit callable."""

    def __init__(self, T):
        import jax
        from jax.sharding import Mesh, PartitionSpec
        from jax.experimental.shard_map import shard_map
        from concourse import bass2jax
        from concourse.bass2jax import _bass_exec_p, partition_id_tensor, install_neuronx_cc_hook

        self.T = T
        nc = build_program(T)
        self.nc = nc
        install_neuronx_cc_hook()

        partition_name = nc.partition_id_tensor.name if nc.partition_id_tensor else None
        in_names, out_names, out_avals = [], [], []
        zero_outs = []
        for alloc in nc.m.functions[0].allocations:
            if not isinstance(alloc, mybir.MemoryLocationSet):
                continue
            name = alloc.memorylocations[0].name
            if alloc.kind == "ExternalInput":
                if name != partition_name:
                    in_names.append(name)
            elif alloc.kind == "ExternalOutput":
                shape = tuple(alloc.tensor_shape)
                dtype = mybir.dt.np(alloc.dtype)
                out_names.append(name)
                out_avals.append(jax.core.ShapedArray(shape, dtype))
                zero_outs.append(np.zeros(shape, dtype))
        self.n_params = len(in_names)
        self.param_names = list(in_names)
        self.out_names = list(out_names)
        self.zero_outs = zero_outs
        in_names = in_names + out_names
        if partition_name is not None:
            in_names.append(partition_name)

        def _body(*args):
            operands = list(args)
            if partition_name is not None:
                operands.append(partition_id_tensor())
            outs = _bass_exec_p.bind(
                *operands,
                out_avals=tuple(out_avals),
                in_names=tuple(in_names),
                out_names=tuple(out_names),
                lowering_input_output_aliases=(),
                sim_require_finite=True,
                sim_require_nnan=True,
                nc=nc,
            )
            return tuple(outs)

        devices = jax.devices()[:NCORES]
        self.mesh = Mesh(np.asarray(devices), ("core",))
        n_outs = len(out_names)
        in_specs = (PartitionSpec("core"),) * (self.n_params + n_outs)
        out_specs = (PartitionSpec("core"),) * n_outs
        donate = tuple(range(self.n_params, self.n_params + n_outs))
        self.sharded = jax.jit(
            shard_map(_body, mesh=self.mesh, in_specs=in_specs, out_specs=out_specs,
                      check_rep=False),
            donate_argnums=donate, keep_unused=True,
        )
        self.out_avals = out_avals
        self._concat_in = None

    def stage_inputs(self, in_maps):
        import jax
        from jax.sharding import NamedSharding, PartitionSpec

        sh = NamedSharding(self.mesh, PartitionSpec("core"))
        self._concat_in = [
            jax.device_put(
                np.concatenate(
                    [np.asarray(in_maps[c][n]) for c in range(NCORES)], axis=0
                ),
                sh,
            )
            for n in self.param_names
        ]
        self._concat_in = jax.block_until_ready(self._concat_in)

    def run(self):
        import jax
        import jax.numpy as jnp
        from jax.sharding import NamedSharding, PartitionSpec

        sh = NamedSharding(self.mesh, PartitionSpec("core"))
        zeros = [
            jnp.zeros((NCORES * z.shape[0], *z.shape[1:]), z.dtype, device=sh)
            for z in self.zero_outs
        ]
        out = self.sharded(*self._concat_in, *zeros)
        out = jax.block_until_ready(out)
        return [
            {
                name: np.asarray(out[i]).reshape(NCORES, *self.out_avals[i].shape)[c]
                for i, name in enumerate(self.out_names)
            }
            for c in range(NCORES)
        ]


_CACHE = {}


def _get_compiled(T):
    if T not in _CACHE:
        _CACHE[T] = CompiledKernel(T)
    return _CACHE[T]


def assemble(T, results):
    B = B_LOC * 4
    outputs = np.empty((T, B, H), np.float32)
    hidden = np.empty((2, B, H), np.float32)
    half = T // 2
    for c in range(NCORES):
        d, s = c // 4, c % 4
        bsl = slice(B_LOC * s, B_LOC * (s + 1))
        oc = results[c]["outc"]                       # [T, KT, P, B_LOC]
        arr = oc.transpose(0, 3, 1, 2).reshape(T, B_LOC, H)
        if d == 0:
            outputs[:half, bsl] = arr[:half]
        else:
            outputs[half:, bsl] = arr[half:]
        hl = results[c]["hlast"]                      # [KT, P, B_LOC]
        hidden[d, bsl] = hl.transpose(2, 0, 1).reshape(B_LOC, H)
    return outputs, hidden


def kernel(sequence, input_lengths, embedding,
           Wih_f, Whh_f, bih_f, bhh_f, Wih_b, Whh_b, bih_b, bhh_b):
    T = int(np.asarray(sequence).shape[0])
    ck = _get_compiled(T)
    in_maps = host_prep(T, sequence, input_lengths, embedding,
                        Wih_f, Whh_f, bih_f, bhh_f, Wih_b, Whh_b, bih_b, bhh_b)
    ck.stage_inputs(in_maps)
    results = ck.run()
    return assemble(T, results)
